# revision 21
# baseline (speedup 1.0000x reference)
"""PointTransformerLayer on 8 Trainium2 NeuronCores.

Sharding: head-parallel attention (H=8, core c owns head c for both batches),
megatron-style row-sharded proj with a ReduceScatter over cores, then
token-sharded MLP (each core gets 512 tokens of each batch).

Layouts: all activations feeding matmuls are kept feature-major ("transposed",
[feat, token]) so the PE contracts over features; LayerNorm stats are computed
token-major and the normalize+scale is folded into the PE-transpose epilogue.
Softmax skips max-subtraction (max logit ~38.6, exp fits fp32 easily) and gets
its denominators for free from a ones-column appended to V.
"""

import numpy as np
import ml_dtypes

import concourse.bass as bass
import concourse.tile as tile
from concourse import bacc, masks, mybir
from concourse.bass_utils import run_bass_kernel_spmd

F32 = mybir.dt.float32
BF16 = mybir.dt.bfloat16
AF = mybir.ActivationFunctionType
ALU = mybir.AluOpType

N_CORES = 8
B, N, D = 2, 4096, 512
H, HD = 8, 64
SCALE = HD ** -0.5
MLP_HID = 4 * D
EPS = 1e-5
NT = B * N              # 8192 tokens total
CHUNK = 512             # token chunk for projections / q-blocks
NCH = NT // CHUNK       # 16 chunks (8 per batch)
KT = N // 128           # 32 k-tiles per batch
FT = D // 128           # 4 feature tiles
MT = MLP_HID // 128     # 16 mlp hidden tiles
SHARD = N // N_CORES    # 512 tokens per batch per core


def _emit(nc, tc, io):
    ctx_pools = []

    def pool(name, bufs, space="SBUF"):
        p = tc.tile_pool(name=name, bufs=bufs, space=space)
        ctx_pools.append(p)
        return p.__enter__()

    consts = pool("consts", 1)
    persist = pool("persist", 1)
    dram = pool("dram", 1, "DRAM")
    p_x = pool("p_x", 2)
    p_st = pool("p_st", 4)
    p_h = pool("p_h", 4)
    p_hT = pool("p_hT", 2)
    p_hidT = pool("p_hidT", 2)
    p_qkb = pool("p_qkb", 2)
    p_vT = pool("p_vT", 2)
    p_pT = pool("p_pT", 3)
    p_oT = pool("p_oT", 2)
    p_pr = pool("p_pr", 2)
    p_mlp = pool("p_mlp", 2)
    p_hg = pool("p_hg", 1)
    p_out = pool("p_out", 2)
    ps_big = pool("ps_big", 2, "PSUM")
    ps_acc = pool("ps_acc", 2, "PSUM")
    ps_s2 = pool("ps_s2", 2, "PSUM")

    # ---- constants / weights into SBUF ----
    ident_bf = consts.tile([128, 128], BF16)
    masks.make_identity(nc, ident_bf)
    ident_f32 = consts.tile([128, 128], F32)
    masks.make_identity(nc, ident_f32)
    ones64 = consts.tile([1, 64], F32)
    nc.vector.memset(ones64, 1.0)
    eps_sb = consts.tile([128, 1], F32)
    nc.vector.memset(eps_sb, EPS)

    def load(name, shape, dtype):
        t = consts.tile(shape, dtype, name=f"c_{name}")
        nc.sync.dma_start(out=t, in_=io[name])
        return t

    wqk_sb = load("wqk", [128, FT, 128], BF16)
    wv_sb = load("wv", [128, FT, 64], BF16)
    bq_sb = load("bq", [64, 1], F32)
    bk_sb = load("bk", [64, 1], F32)
    bv_sb = load("bv", [64, 1], F32)
    pe_w1_sb = load("pe_w1", [3, D], BF16)
    pe_b1_sb = load("pe_b1r", [128, FT], F32)
    pe_w2_sb = load("pe_w2c", [128, FT, 64], BF16)
    pe_b2_sb = load("pe_b2c", [64, 1], F32)
    projw_sb = load("projw", [64, D], BF16)
    w1_sb = load("w1", [128, FT, MLP_HID], BF16)
    b1r_sb = load("b1r", [128, MT], F32)
    w2_sb = load("w2", [128, MT, D], BF16)
    posT_sb = p_hg.tile([3, NT], BF16, name="c_posT", tag="hg")
    nc.sync.dma_start(out=posT_sb, in_=io["posT"])
    xs_ap = io["xs"]
    x_ap = io["x"]
    out_ap = io["out"]

    def part_bcast(ap, p=128):
        return bass.AP(tensor=ap.tensor, offset=ap.offset,
                       ap=[[0, p], *[list(d) for d in ap.ap]])

    projb_bc = consts.tile([128, D], F32)
    nc.sync.dma_start(out=projb_bc, in_=part_bcast(io["projb"]))
    b2_bc = consts.tile([128, D], F32)
    nc.sync.dma_start(out=b2_bc, in_=part_bcast(io["b2"]))

    # ---- persistent activations ----
    qT_sb = persist.tile([64, NT], BF16)
    kT_sb = persist.tile([64, NT], BF16)
    V_sb = persist.tile([128, NT // 128, 65], BF16)  # token-major V + ones col
    z_sb = persist.tile([128, 8, D], F32)            # post-attn residual (shard)
    mv_b1 = persist.tile([128, 32, 2], F32)          # hoisted LN1 stats, batch 1
    rin_b1 = persist.tile([128, 32], F32)
    nc.vector.memset(V_sb[:, :, 64:65], 1.0)

    rs_in = [dram.tile([N // 2, D], F32, name=f"rs_in{i}") for i in range(2 * B)]
    rs_out = [dram.tile([N // 2 // N_CORES, D], F32, name=f"rs_out{i}")
              for i in range(2 * B)]

    # ============ stage 1a: positional embedding (all Gelu up front) =========
    def emit_pe(ci):
        sl = slice(ci * CHUNK, (ci + 1) * CHUNK)
        hidT = p_hidT.tile([128, FT, CHUNK], BF16, name="hidT")
        for ft in range(FT):
            hid_ps = ps_big.tile([128, CHUNK], F32, name="hid_ps", tag="big")
            nc.tensor.matmul(hid_ps, pe_w1_sb[:, ft * 128:(ft + 1) * 128],
                             posT_sb[:, sl], start=True, stop=True)
            nc.scalar.activation(hidT[:, ft, :], hid_ps, AF.Gelu,
                                 bias=pe_b1_sb[:, ft:ft + 1], scale=1.0)
        peT_ps = ps_acc.tile([64, CHUNK], F32, name="peT_ps", tag="acc")
        for ft in range(FT):
            nc.tensor.matmul(peT_ps, pe_w2_sb[:, ft, :], hidT[:, ft, :],
                             start=(ft == 0), stop=(ft == FT - 1))
        nc.vector.tensor_scalar_add(qT_sb[:, sl], peT_ps, pe_b2_sb)
        nc.vector.tensor_scalar_add(kT_sb[:, sl], peT_ps, pe_b2_sb)

    # ============ stage 1b-pre: LN1 stats only (hoists ACT Sqrt) =============
    def emit_ln_stats(ci):
        for tt in range(4):
            g = ci * 4 + tt
            gi = g - 32
            x_sb = p_x.tile([128, D], F32, name="x_sb")
            nc.sync.dma_start(out=x_sb, in_=x_ap[g * 128:(g + 1) * 128, :])
            stats = p_st.tile([128, 6], F32, name="stats")
            nc.vector.bn_stats(stats, x_sb)
            nc.vector.bn_aggr(mv_b1[:, gi, :], stats)
            std = p_st.tile([128, 1], F32, name="std")
            nc.scalar.activation(std, mv_b1[:, gi, 1:2], AF.Sqrt, bias=eps_sb,
                                 scale=1.0)
            nc.vector.reciprocal(rin_b1[:, gi:gi + 1], std)

    # ============ stage 1b: LN1 + transpose + qkvT ===========================
    # Emits DVE/DMA prep now; returns PE-work thunks for fine-grained
    # interleaving into the attention k-loop (keeps PE duty high for HAM).
    def emit_ln_qkv_units(ci):
        sl = slice(ci * CHUNK, (ci + 1) * CHUNK)
        hT = p_hT.tile([128, FT, CHUNK], BF16, name="hT")
        h_tiles = []
        for tt in range(4):
            g = ci * 4 + tt
            x_sb = p_x.tile([128, D], F32, name="x_sb")
            nc.sync.dma_start(out=x_sb, in_=x_ap[g * 128:(g + 1) * 128, :])
            if ci < 8:
                stats = p_st.tile([128, 6], F32, name="stats")
                nc.vector.bn_stats(stats, x_sb)
                mv = p_st.tile([128, 2], F32, name="mv")
                nc.vector.bn_aggr(mv, stats)
                std = p_st.tile([128, 1], F32, name="std")
                nc.scalar.activation(std, mv[:, 1:2], AF.Sqrt, bias=eps_sb,
                                     scale=1.0)
                rin = p_st.tile([128, 1], F32, name="rin")
                nc.vector.reciprocal(rin, std)
                mu, ri = mv[:, 0:1], rin
            else:
                gi = g - 32
                mu, ri = mv_b1[:, gi, 0:1], rin_b1[:, gi:gi + 1]
            h_sb = p_h.tile([128, D], BF16, name="h_sb")
            nc.vector.tensor_scalar(h_sb, x_sb, mu, ri,
                                    op0=ALU.subtract, op1=ALU.mult)
            h_tiles.append(h_sb)

        def tp_unit(tt, h_sb):
            for ft in range(FT):
                tp_ps = ps_s2.tile([128, 128], BF16, name="tp_ps", tag="s2")
                nc.tensor.transpose(tp_ps, h_sb[:, ft * 128:(ft + 1) * 128],
                                    ident_bf)
                nc.vector.tensor_copy(hT[:, ft, tt * 128:(tt + 1) * 128], tp_ps)

        def qk_unit():
            qk_ps = ps_big.tile([128, CHUNK], F32, name="qk_ps", tag="big")
            for ft in range(FT):
                nc.tensor.matmul(qk_ps, wqk_sb[:, ft, :], hT[:, ft, :],
                                 start=(ft == 0), stop=(ft == FT - 1))
            bq_tmp = p_qkb.tile([64, CHUNK], BF16, name="bq_tmp", tag="qkb")
            nc.vector.tensor_scalar_add(bq_tmp, qk_ps[0:64, :], bq_sb)
            nc.vector.tensor_add(qT_sb[:, sl], qT_sb[:, sl], bq_tmp)
            bk_tmp = p_qkb.tile([64, CHUNK], BF16, name="bk_tmp", tag="qkb")
            nc.vector.tensor_scalar_add(bk_tmp, qk_ps[64:128, :], bk_sb)
            nc.vector.tensor_add(kT_sb[:, sl], kT_sb[:, sl], bk_tmp)

        def v_unit():
            v_ps = ps_acc.tile([64, CHUNK], F32, name="v_ps", tag="acc")
            for ft in range(FT):
                nc.tensor.matmul(v_ps, wv_sb[:, ft, :], hT[:, ft, :],
                                 start=(ft == 0), stop=(ft == FT - 1))
            vT = p_vT.tile([64, CHUNK], BF16, name="vT")
            nc.vector.tensor_scalar_add(vT, v_ps, bv_sb)
            for ss in range(4):
                vtp_ps = ps_s2.tile([128, 64], BF16, name="vtp_ps", tag="s2")
                nc.tensor.transpose(vtp_ps, vT[:, ss * 128:(ss + 1) * 128],
                                    ident_bf[0:64, 0:64])
                nc.vector.tensor_copy(V_sb[:, ci * 4 + ss, 0:64], vtp_ps)

        units = [lambda tt=tt, h=h_tiles[tt]: tp_unit(tt, h) for tt in range(4)]
        units += [qk_unit, v_unit]
        return units

    def emit_ln_qkv(ci):
        for u in emit_ln_qkv_units(ci):
            u()

    # ============ stage 2: attention + proj partial ==========================
    def emit_attention(b, fillers=None, unit_gen=None):
        base = b * N
        for qc in range(N // CHUNK):
            units = list(unit_gen(qc)) if unit_gen else []
            qsl = slice(base + qc * CHUNK, base + (qc + 1) * CHUNK)
            oT_ps = ps_acc.tile([65, CHUNK], F32, name="oT_ps", tag="acc")
            for kp in range(KT // 2):
                st2_ps = ps_s2.tile([128, 2 * CHUNK], F32, name="st2_ps", tag="s2")
                for j in range(2):
                    kt = kp * 2 + j
                    ksl = slice(base + kt * 128, base + (kt + 1) * 128)
                    nc.tensor.matmul(st2_ps[:, j * CHUNK:(j + 1) * CHUNK],
                                     kT_sb[:, ksl], qT_sb[:, qsl],
                                     start=True, stop=True)
                pT = p_pT.tile([128, 2 * CHUNK], BF16, name="pT")
                nc.scalar.activation(pT, st2_ps, AF.Exp, scale=SCALE)
                for j in range(2):
                    kt = kp * 2 + j
                    nc.tensor.matmul(oT_ps, V_sb[:, b * KT + kt, :],
                                     pT[:, j * CHUNK:(j + 1) * CHUNK],
                                     start=(kt == 0), stop=(kt == KT - 1))
                if kp % 2 == 1 and units:
                    units.pop(0)()
            o65_sb = p_oT.tile([65, CHUNK], F32, name="o65_sb", tag="o65")
            nc.vector.tensor_copy(o65_sb, oT_ps)
            dcol_ps = ps_big.tile([128, 4], F32, name="dcol_ps", tag="big")
            for ss in range(4):
                nc.tensor.transpose(dcol_ps[:, ss:ss + 1],
                                    o65_sb[64:65, ss * 128:(ss + 1) * 128],
                                    ident_f32[64:65, 64:65])
            rcol_sb = p_st.tile([128, 4], F32, name="rcol_sb")
            nc.vector.reciprocal(rcol_sb, dcol_ps)
            rrow_ps = ps_big.tile([1, CHUNK], F32, name="rrow_ps", tag="big")
            for ss in range(4):
                nc.tensor.transpose(rrow_ps[:, ss * 128:(ss + 1) * 128],
                                    rcol_sb[:, ss:ss + 1], ident_f32)
            recip = p_st.tile([1, CHUNK], F32, name="recip")
            nc.vector.tensor_copy(recip, rrow_ps)
            rb_ps = ps_big.tile([64, CHUNK], F32, name="rb_ps", tag="big")
            nc.tensor.matmul(rb_ps, ones64, recip, start=True, stop=True)
            oT_sb = p_oT.tile([64, CHUNK], BF16, name="oT_sb")
            nc.vector.tensor_mul(oT_sb, o65_sb[0:64, :], rb_ps)
            for ss in range(4):
                pr_ps = ps_big.tile([128, D], F32, name="pr_ps", tag="big")
                nc.tensor.matmul(pr_ps, oT_sb[:, ss * 128:(ss + 1) * 128], projw_sb,
                                 start=True, stop=True)
                pr_sb = p_pr.tile([128, D], F32, name="pr_sb")
                nc.vector.tensor_copy(pr_sb, pr_ps)
                r0 = (qc % 4) * CHUNK + ss * 128
                nc.sync.dma_start(out=rs_in[2 * b + qc // 4][r0:r0 + 128, :],
                                  in_=pr_sb)
            if qc == 3 or qc == 7:
                half = qc // 4
                nc.gpsimd.collective_compute(
                    "ReduceScatter", ALU.add,
                    replica_groups=[list(range(N_CORES))],
                    ins=[rs_in[2 * b + half].opt()],
                    outs=[rs_out[2 * b + half].opt()],
                )
            if fillers and qc in fillers:
                fillers[qc]()

    # =================== stage 3: residual + LN2 + MLP (token shard) =========
    def emit_mlp_ln(b):
        h2T = p_mlp.tile([128, FT, CHUNK], BF16, name="h2T")
        for tt in range(4):
            zi = b * 4 + tt
            xr = p_x.tile([128, D], F32, name="xr")
            nc.sync.dma_start(out=xr, in_=xs_ap[zi * 128:(zi + 1) * 128, :])
            rso = p_pr.tile([128, D], F32, name="rso")
            nc.sync.dma_start(
                out=rso,
                in_=rs_out[2 * b + tt // 2][(tt % 2) * 128:(tt % 2 + 1) * 128, :])
            nc.vector.tensor_add(z_sb[:, zi, :], xr, rso)
            nc.vector.tensor_add(z_sb[:, zi, :], z_sb[:, zi, :], projb_bc)
            stats = p_st.tile([128, 6], F32, name="stats2")
            nc.vector.bn_stats(stats, z_sb[:, zi, :])
            mv = p_st.tile([128, 2], F32, name="mv2")
            nc.vector.bn_aggr(mv, stats)
            std = p_st.tile([128, 1], F32, name="std2")
            nc.scalar.activation(std, mv[:, 1:2], AF.Sqrt, bias=eps_sb, scale=1.0)
            rin = p_st.tile([128, 1], F32, name="rin2")
            nc.vector.reciprocal(rin, std)
            h2 = p_h.tile([128, D], BF16, name="h2")
            nc.vector.tensor_scalar(h2, z_sb[:, zi, :], mv[:, 0:1], rin,
                                    op0=ALU.subtract, op1=ALU.mult)
            for ft in range(FT):
                tp_ps = ps_s2.tile([128, 128], BF16, name="tp2_ps", tag="s2")
                nc.tensor.transpose(tp_ps, h2[:, ft * 128:(ft + 1) * 128], ident_bf)
                nc.vector.tensor_copy(h2T[:, ft, tt * 128:(tt + 1) * 128], tp_ps)
        return h2T

    def emit_mlp_ffn(b, h2T):
        hg = p_hg.tile([128, MT, CHUNK], BF16, name="hg", tag="hg")
        for mt in range(MT):
            hid_ps = ps_big.tile([128, CHUNK], F32, name="mhid_ps", tag="big")
            for ft in range(FT):
                nc.tensor.matmul(hid_ps, w1_sb[:, ft, mt * 128:(mt + 1) * 128],
                                 h2T[:, ft, :], start=(ft == 0), stop=(ft == FT - 1))
            nc.scalar.activation(hg[:, mt, :], hid_ps, AF.Gelu,
                                 bias=b1r_sb[:, mt:mt + 1], scale=1.0)
        outT = p_mlp.tile([128, FT, CHUNK], F32, name="outT")
        for ft in range(FT):
            o2_ps = ps_big.tile([128, CHUNK], F32, name="o2_ps", tag="big")
            for mt in range(MT):
                nc.tensor.matmul(o2_ps, w2_sb[:, mt, ft * 128:(ft + 1) * 128],
                                 hg[:, mt, :], start=(mt == 0), stop=(mt == MT - 1))
            nc.vector.tensor_copy(outT[:, ft, :], o2_ps)
        for tt in range(4):
            out_sb = p_out.tile([128, D], F32, name="out_sb")
            for ft in range(FT):
                tp_ps = ps_s2.tile([128, 128], F32, name="tpo_ps", tag="s2")
                nc.tensor.transpose(tp_ps, outT[:, ft, tt * 128:(tt + 1) * 128],
                                    ident_f32)
                csl = slice(ft * 128, (ft + 1) * 128)
                nc.vector.tensor_add(out_sb[:, csl], tp_ps, z_sb[:, b * 4 + tt, csl])
                nc.vector.tensor_add(out_sb[:, csl], out_sb[:, csl], b2_bc[:, csl])
            r0 = b * SHARD + tt * 128
            nc.sync.dma_start(out=out_ap[r0:r0 + 128, :], in_=out_sb)

    for ci in range(16):
        emit_pe(ci)
    for ci in range(8, 16):
        emit_ln_stats(ci)
    for ci in range(8):
        emit_ln_qkv(ci)
    emit_attention(0, unit_gen=lambda qc: emit_ln_qkv_units(8 + qc))
    state = {}
    emit_attention(1, {
        4: lambda: state.update(h2T0=emit_mlp_ln(0)),
        6: lambda: emit_mlp_ffn(0, state["h2T0"]),
    })
    h2T1 = emit_mlp_ln(1)
    emit_mlp_ffn(1, h2T1)

    for p in reversed(ctx_pools):
        p.__exit__(None, None, None)


_INPUT_SPECS = {
    "x": ([NT, D], F32),
    "xs": ([2 * SHARD, D], F32),
    "posT": ([3, NT], BF16),
    "wqk": ([128, FT, 128], BF16),
    "wv": ([128, FT, 64], BF16),
    "bq": ([64, 1], F32),
    "bk": ([64, 1], F32),
    "bv": ([64, 1], F32),
    "pe_w1": ([3, D], BF16),
    "pe_b1r": ([128, FT], F32),
    "pe_w2c": ([128, FT, 64], BF16),
    "pe_b2c": ([64, 1], F32),
    "projw": ([64, D], BF16),
    "projb": ([D], F32),
    "w1": ([128, FT, MLP_HID], BF16),
    "b1r": ([128, MT], F32),
    "w2": ([128, MT, D], BF16),
    "b2": ([D], F32),
}

_NC_CACHE = []


def build_module():
    if _NC_CACHE:
        return _NC_CACHE[0]
    nc = bacc.Bacc("TRN2", target_bir_lowering=False, debug=False,
                   num_devices=N_CORES)
    io = {}
    for name, (shape, dt) in _INPUT_SPECS.items():
        io[name] = nc.dram_tensor(name, shape, dt, kind="ExternalInput").ap()
    io["out"] = nc.dram_tensor("out", [2 * SHARD, D], F32,
                               kind="ExternalOutput").ap()
    with tile.TileContext(nc) as tc:
        _emit(nc, tc, io)
    nc.compile()
    _NC_CACHE.append(nc)
    return nc


def _feat_tiles(a, inner):
    """[D_total, inner] -> [128, D_total//128, inner] (partition-major tiles)."""
    d = a.shape[0]
    return np.ascontiguousarray(
        a.reshape(d // 128, 128, inner).transpose(1, 0, 2))


def _prep_inputs(c, x, pos, qkv_w, qkv_b, proj_w, proj_b, pe_w1, pe_b1, pe_w2,
                 pe_b2, mlp_w1, mlp_b1, mlp_w2, mlp_b2, n1_g, n1_b, n2_g, n2_b):
    bf = ml_dtypes.bfloat16
    f32 = np.float32
    x_flat = np.ascontiguousarray(x.reshape(NT, D).astype(f32))
    cs = slice(c * HD, (c + 1) * HD)
    g1 = n1_g.astype(np.float64)[:, None]
    wq = qkv_w[:, cs] * g1
    wk = qkv_w[:, D + c * HD:D + (c + 1) * HD] * g1
    wv = qkv_w[:, 2 * D + c * HD:2 * D + (c + 1) * HD] * g1
    bq_f = qkv_b[cs] + n1_b @ qkv_w[:, cs]
    bk_f = qkv_b[D + c * HD:D + (c + 1) * HD] + n1_b @ qkv_w[:, D + c * HD:D + (c + 1) * HD]
    bv_f = qkv_b[2 * D + c * HD:2 * D + (c + 1) * HD] + n1_b @ qkv_w[:, 2 * D + c * HD:2 * D + (c + 1) * HD]
    w1_f = mlp_w1 * n2_g.astype(np.float64)[:, None]
    b1_f = mlp_b1 + n2_b @ mlp_w1
    xs = np.concatenate(
        [x_flat[b * N + (tt // 2) * (N // 2) + c * 256 + (tt % 2) * 128:][:128]
         for b in range(B) for tt in range(4)], axis=0)
    per_part = lambda v: np.ascontiguousarray(
        v.reshape(-1, 128).T.astype(f32))
    return {
        "x": x_flat,
        "xs": np.ascontiguousarray(xs),
        "posT": np.ascontiguousarray(pos.reshape(NT, 3).T.astype(bf)),
        "wqk": _feat_tiles(np.concatenate([wq, wk], axis=1).astype(bf), 128),
        "wv": _feat_tiles(wv.astype(bf), HD),
        "bq": bq_f.astype(f32).reshape(HD, 1),
        "bk": bk_f.astype(f32).reshape(HD, 1),
        "bv": bv_f.astype(f32).reshape(HD, 1),
        "pe_w1": np.ascontiguousarray(pe_w1.astype(bf)),
        "pe_b1r": per_part(pe_b1),
        "pe_w2c": _feat_tiles(pe_w2[:, cs].astype(bf), HD),
        "pe_b2c": pe_b2[cs].astype(f32).reshape(HD, 1),
        "projw": np.ascontiguousarray(proj_w[cs, :].astype(bf)),
        "projb": proj_b.astype(f32),
        "w1": _feat_tiles(w1_f.astype(bf), MLP_HID),
        "b1r": per_part(b1_f),
        "w2": _feat_tiles(mlp_w2.astype(bf), D),
        "b2": mlp_b2.astype(f32),
    }


def kernel(**inputs):
    nc = build_module()
    in_maps = [_prep_inputs(c, **{k: np.asarray(v) for k, v in inputs.items()})
               for c in range(N_CORES)]
    res = run_bass_kernel_spmd(nc, in_maps, core_ids=list(range(N_CORES)),
                               trace=False)
    out = np.empty((B, N, D), np.float32)
    for c in range(N_CORES):
        o = res.results[c]["out"]
        for b in range(B):
            for tt in range(4):
                g0 = (tt // 2) * (N // 2) + c * 256 + (tt % 2) * 128
                out[b, g0:g0 + 128, :] = o[b * SHARD + tt * 128:][:128, :]
    return out


# revision 22
# speedup vs baseline: 1.1551x; 1.1551x over previous
"""PointTransformerLayer on 8 Trainium2 NeuronCores.

Sharding: head-parallel attention (H=8, core c owns head c for both batches),
megatron-style row-sharded proj with a ReduceScatter over cores, then
token-sharded MLP (each core gets 512 tokens of each batch).

Layouts: all activations feeding matmuls are kept feature-major ("transposed",
[feat, token]) so the PE contracts over features; LayerNorm stats are computed
token-major and the normalize+scale is folded into the PE-transpose epilogue.
Softmax skips max-subtraction (max logit ~38.6, exp fits fp32 easily) and gets
its denominators for free from a ones-column appended to V.
"""

import numpy as np
import ml_dtypes

import concourse.bass as bass
import concourse.tile as tile
from concourse.tile import add_dep_helper
from concourse import bacc, masks, mybir
from concourse.bass_utils import run_bass_kernel_spmd

F32 = mybir.dt.float32
BF16 = mybir.dt.bfloat16
AF = mybir.ActivationFunctionType
ALU = mybir.AluOpType

N_CORES = 8
B, N, D = 2, 4096, 512
H, HD = 8, 64
SCALE = HD ** -0.5
MLP_HID = 4 * D
EPS = 1e-5
NT = B * N              # 8192 tokens total
CHUNK = 512             # token chunk for projections / q-blocks
NCH = NT // CHUNK       # 16 chunks (8 per batch)
KT = N // 128           # 32 k-tiles per batch
FT = D // 128           # 4 feature tiles
MT = MLP_HID // 128     # 16 mlp hidden tiles
SHARD = N // N_CORES    # 512 tokens per batch per core


def _emit(nc, tc, io):
    ctx_pools = []
    act_groups = {"gelu_a": [], "sqrt_head": [], "exp0": []}

    def pool(name, bufs, space="SBUF"):
        p = tc.tile_pool(name=name, bufs=bufs, space=space)
        ctx_pools.append(p)
        return p.__enter__()

    consts = pool("consts", 1)
    persist = pool("persist", 1)
    dram = pool("dram", 1, "DRAM")
    p_x = pool("p_x", 2)
    p_st = pool("p_st", 4)
    p_h = pool("p_h", 4)
    p_hT = pool("p_hT", 2)
    p_hidT = pool("p_hidT", 2)
    p_qkb = pool("p_qkb", 2)
    p_vT = pool("p_vT", 2)
    p_pT = pool("p_pT", 3)
    p_oT = pool("p_oT", 2)
    p_pr = pool("p_pr", 2)
    p_mlp = pool("p_mlp", 2)
    p_hg = pool("p_hg", 1)
    p_out = pool("p_out", 2)
    ps_big = pool("ps_big", 2, "PSUM")
    ps_acc = pool("ps_acc", 2, "PSUM")
    ps_s2 = pool("ps_s2", 2, "PSUM")

    # ---- constants / weights into SBUF ----
    ident_bf = consts.tile([128, 128], BF16)
    masks.make_identity(nc, ident_bf)
    ident_f32 = consts.tile([128, 128], F32)
    masks.make_identity(nc, ident_f32)
    ones64 = consts.tile([1, 64], F32)
    nc.vector.memset(ones64, 1.0)
    eps_sb = consts.tile([128, 1], F32)
    nc.vector.memset(eps_sb, EPS)

    def load(name, shape, dtype):
        t = consts.tile(shape, dtype, name=f"c_{name}")
        nc.sync.dma_start(out=t, in_=io[name])
        return t

    wqk_sb = load("wqk", [128, FT, 128], BF16)
    wv_sb = load("wv", [128, FT, 64], BF16)
    bq_sb = load("bq", [64, 1], F32)
    bk_sb = load("bk", [64, 1], F32)
    bv_sb = load("bv", [64, 1], F32)
    pe_w1_sb = load("pe_w1", [3, D], BF16)
    pe_b1_sb = load("pe_b1r", [128, FT], F32)
    pe_w2_sb = load("pe_w2c", [128, FT, 64], BF16)
    pe_b2_sb = load("pe_b2c", [64, 1], F32)
    projw_sb = load("projw", [64, D], BF16)
    w1_sb = load("w1", [128, FT, MLP_HID], BF16)
    b1r_sb = load("b1r", [128, MT], F32)
    w2_sb = load("w2", [128, MT, D], BF16)
    posT_sb = p_hg.tile([3, NT], BF16, name="c_posT", tag="hg")
    nc.sync.dma_start(out=posT_sb, in_=io["posT"])
    xs_ap = io["xs"]
    x_ap = io["x"]
    out_ap = io["out"]

    def part_bcast(ap, p=128):
        return bass.AP(tensor=ap.tensor, offset=ap.offset,
                       ap=[[0, p], *[list(d) for d in ap.ap]])

    projb_bc = consts.tile([128, D], F32)
    nc.sync.dma_start(out=projb_bc, in_=part_bcast(io["projb"]))
    b2_bc = consts.tile([128, D], F32)
    nc.sync.dma_start(out=b2_bc, in_=part_bcast(io["b2"]))

    # ---- persistent activations ----
    qT_sb = persist.tile([64, NT], BF16)
    kT_sb = persist.tile([64, NT], BF16)
    V_sb = persist.tile([128, NT // 128, 65], BF16)  # token-major V + ones col
    z_sb = persist.tile([128, 8, D], F32)            # post-attn residual (shard)
    mv_b1 = persist.tile([128, 32, 2], F32)          # hoisted LN1 stats, batch 1
    rin_b1 = persist.tile([128, 32], F32)
    nc.vector.memset(V_sb[:, :, 64:65], 1.0)

    rs_in = [dram.tile([N // 2, D], F32, name=f"rs_in{i}") for i in range(2 * B)]
    rs_out = [dram.tile([N // 2 // N_CORES, D], F32, name=f"rs_out{i}")
              for i in range(2 * B)]

    # ============ stage 1a: positional embedding (all Gelu up front) =========
    def emit_pe(ci):
        sl = slice(ci * CHUNK, (ci + 1) * CHUNK)
        hidT = p_hidT.tile([128, FT, CHUNK], BF16, name="hidT")
        for ft in range(FT):
            hid_ps = ps_big.tile([128, CHUNK], F32, name="hid_ps", tag="big")
            nc.tensor.matmul(hid_ps, pe_w1_sb[:, ft * 128:(ft + 1) * 128],
                             posT_sb[:, sl], start=True, stop=True)
            act_groups["gelu_a"].append(
                nc.scalar.activation(hidT[:, ft, :], hid_ps, AF.Gelu,
                                     bias=pe_b1_sb[:, ft:ft + 1], scale=1.0))
        peT_ps = ps_acc.tile([64, CHUNK], F32, name="peT_ps", tag="acc")
        for ft in range(FT):
            nc.tensor.matmul(peT_ps, pe_w2_sb[:, ft, :], hidT[:, ft, :],
                             start=(ft == 0), stop=(ft == FT - 1))
        nc.vector.tensor_scalar_add(qT_sb[:, sl], peT_ps, pe_b2_sb)
        nc.vector.tensor_scalar_add(kT_sb[:, sl], peT_ps, pe_b2_sb)

    # ============ stage 1b-pre: LN1 stats only (hoists ACT Sqrt) =============
    def emit_ln_stats(ci):
        for tt in range(4):
            g = ci * 4 + tt
            gi = g - 32
            x_sb = p_x.tile([128, D], F32, name="x_sb")
            nc.sync.dma_start(out=x_sb, in_=x_ap[g * 128:(g + 1) * 128, :])
            stats = p_st.tile([128, 6], F32, name="stats")
            nc.vector.bn_stats(stats, x_sb)
            nc.vector.bn_aggr(mv_b1[:, gi, :], stats)
            std = p_st.tile([128, 1], F32, name="std")
            act_groups["sqrt_head"].append(
                nc.scalar.activation(std, mv_b1[:, gi, 1:2], AF.Sqrt,
                                     bias=eps_sb, scale=1.0))
            nc.vector.reciprocal(rin_b1[:, gi:gi + 1], std)

    # ============ stage 1b: LN1 + transpose + qkvT ===========================
    # Emits DVE/DMA prep now; returns PE-work thunks for fine-grained
    # interleaving into the attention k-loop (keeps PE duty high for HAM).
    def emit_ln_qkv_units(ci):
        sl = slice(ci * CHUNK, (ci + 1) * CHUNK)
        hT = p_hT.tile([128, FT, CHUNK], BF16, name="hT")
        h_tiles = []
        for tt in range(4):
            g = ci * 4 + tt
            x_sb = p_x.tile([128, D], F32, name="x_sb")
            nc.sync.dma_start(out=x_sb, in_=x_ap[g * 128:(g + 1) * 128, :])
            if ci < 8:
                stats = p_st.tile([128, 6], F32, name="stats")
                nc.vector.bn_stats(stats, x_sb)
                mv = p_st.tile([128, 2], F32, name="mv")
                nc.vector.bn_aggr(mv, stats)
                std = p_st.tile([128, 1], F32, name="std")
                act_groups["sqrt_head"].append(
                    nc.scalar.activation(std, mv[:, 1:2], AF.Sqrt, bias=eps_sb,
                                         scale=1.0))
                rin = p_st.tile([128, 1], F32, name="rin")
                nc.vector.reciprocal(rin, std)
                mu, ri = mv[:, 0:1], rin
            else:
                gi = g - 32
                mu, ri = mv_b1[:, gi, 0:1], rin_b1[:, gi:gi + 1]
            h_sb = p_h.tile([128, D], BF16, name="h_sb")
            nc.vector.tensor_scalar(h_sb, x_sb, mu, ri,
                                    op0=ALU.subtract, op1=ALU.mult)
            h_tiles.append(h_sb)

        def tp_unit(tt, h_sb):
            for ft in range(FT):
                tp_ps = ps_s2.tile([128, 128], BF16, name="tp_ps", tag="s2")
                nc.tensor.transpose(tp_ps, h_sb[:, ft * 128:(ft + 1) * 128],
                                    ident_bf)
                nc.vector.tensor_copy(hT[:, ft, tt * 128:(tt + 1) * 128], tp_ps)

        def qk_unit():
            qk_ps = ps_big.tile([128, CHUNK], F32, name="qk_ps", tag="big")
            for ft in range(FT):
                nc.tensor.matmul(qk_ps, wqk_sb[:, ft, :], hT[:, ft, :],
                                 start=(ft == 0), stop=(ft == FT - 1))
            bq_tmp = p_qkb.tile([64, CHUNK], BF16, name="bq_tmp", tag="qkb")
            nc.vector.tensor_scalar_add(bq_tmp, qk_ps[0:64, :], bq_sb)
            nc.vector.tensor_add(qT_sb[:, sl], qT_sb[:, sl], bq_tmp)
            bk_tmp = p_qkb.tile([64, CHUNK], BF16, name="bk_tmp", tag="qkb")
            nc.vector.tensor_scalar_add(bk_tmp, qk_ps[64:128, :], bk_sb)
            nc.vector.tensor_add(kT_sb[:, sl], kT_sb[:, sl], bk_tmp)

        def v_unit():
            v_ps = ps_acc.tile([64, CHUNK], F32, name="v_ps", tag="acc")
            for ft in range(FT):
                nc.tensor.matmul(v_ps, wv_sb[:, ft, :], hT[:, ft, :],
                                 start=(ft == 0), stop=(ft == FT - 1))
            vT = p_vT.tile([64, CHUNK], BF16, name="vT")
            nc.vector.tensor_scalar_add(vT, v_ps, bv_sb)
            for ss in range(4):
                vtp_ps = ps_s2.tile([128, 64], BF16, name="vtp_ps", tag="s2")
                nc.tensor.transpose(vtp_ps, vT[:, ss * 128:(ss + 1) * 128],
                                    ident_bf[0:64, 0:64])
                nc.vector.tensor_copy(V_sb[:, ci * 4 + ss, 0:64], vtp_ps)

        units = [lambda tt=tt, h=h_tiles[tt]: tp_unit(tt, h) for tt in range(4)]
        units += [qk_unit, v_unit]
        return units

    def emit_ln_qkv(ci):
        for u in emit_ln_qkv_units(ci):
            u()

    # ============ stage 2: attention + proj partial ==========================
    def emit_attention(b, fillers=None, unit_gen=None):
        base = b * N
        for qc in range(N // CHUNK):
            units = list(unit_gen(qc)) if unit_gen else []
            qsl = slice(base + qc * CHUNK, base + (qc + 1) * CHUNK)
            oT_ps = ps_acc.tile([65, CHUNK], F32, name="oT_ps", tag="acc")
            for kp in range(KT // 2):
                st2_ps = ps_s2.tile([128, 2 * CHUNK], F32, name="st2_ps", tag="s2")
                for j in range(2):
                    kt = kp * 2 + j
                    ksl = slice(base + kt * 128, base + (kt + 1) * 128)
                    nc.tensor.matmul(st2_ps[:, j * CHUNK:(j + 1) * CHUNK],
                                     kT_sb[:, ksl], qT_sb[:, qsl],
                                     start=True, stop=True)
                pT = p_pT.tile([128, 2 * CHUNK], BF16, name="pT")
                e = nc.scalar.activation(pT, st2_ps, AF.Exp, scale=SCALE)
                if b == 0:
                    act_groups["exp0"].append(e)
                for j in range(2):
                    kt = kp * 2 + j
                    nc.tensor.matmul(oT_ps, V_sb[:, b * KT + kt, :],
                                     pT[:, j * CHUNK:(j + 1) * CHUNK],
                                     start=(kt == 0), stop=(kt == KT - 1))
                if kp % 2 == 1 and units:
                    units.pop(0)()
            o65_sb = p_oT.tile([65, CHUNK], F32, name="o65_sb", tag="o65")
            nc.vector.tensor_copy(o65_sb, oT_ps)
            dcol_ps = ps_big.tile([128, 4], F32, name="dcol_ps", tag="big")
            for ss in range(4):
                nc.tensor.transpose(dcol_ps[:, ss:ss + 1],
                                    o65_sb[64:65, ss * 128:(ss + 1) * 128],
                                    ident_f32[64:65, 64:65])
            rcol_sb = p_st.tile([128, 4], F32, name="rcol_sb")
            nc.vector.reciprocal(rcol_sb, dcol_ps)
            rrow_ps = ps_big.tile([1, CHUNK], F32, name="rrow_ps", tag="big")
            for ss in range(4):
                nc.tensor.transpose(rrow_ps[:, ss * 128:(ss + 1) * 128],
                                    rcol_sb[:, ss:ss + 1], ident_f32)
            recip = p_st.tile([1, CHUNK], F32, name="recip")
            nc.vector.tensor_copy(recip, rrow_ps)
            rb_ps = ps_big.tile([64, CHUNK], F32, name="rb_ps", tag="big")
            nc.tensor.matmul(rb_ps, ones64, recip, start=True, stop=True)
            oT_sb = p_oT.tile([64, CHUNK], BF16, name="oT_sb")
            nc.vector.tensor_mul(oT_sb, o65_sb[0:64, :], rb_ps)
            for ss in range(4):
                pr_ps = ps_big.tile([128, D], F32, name="pr_ps", tag="big")
                nc.tensor.matmul(pr_ps, oT_sb[:, ss * 128:(ss + 1) * 128], projw_sb,
                                 start=True, stop=True)
                pr_sb = p_pr.tile([128, D], F32, name="pr_sb")
                nc.vector.tensor_copy(pr_sb, pr_ps)
                r0 = (qc % 4) * CHUNK + ss * 128
                nc.sync.dma_start(out=rs_in[2 * b + qc // 4][r0:r0 + 128, :],
                                  in_=pr_sb)
            if qc == 3 or qc == 7:
                half = qc // 4
                nc.gpsimd.collective_compute(
                    "ReduceScatter", ALU.add,
                    replica_groups=[list(range(N_CORES))],
                    ins=[rs_in[2 * b + half].opt()],
                    outs=[rs_out[2 * b + half].opt()],
                )
            if fillers and qc in fillers:
                fillers[qc]()

    # =================== stage 3: residual + LN2 + MLP (token shard) =========
    def emit_mlp_ln(b):
        h2T = p_mlp.tile([128, FT, CHUNK], BF16, name="h2T")
        for tt in range(4):
            zi = b * 4 + tt
            xr = p_x.tile([128, D], F32, name="xr")
            nc.sync.dma_start(out=xr, in_=xs_ap[zi * 128:(zi + 1) * 128, :])
            rso = p_pr.tile([128, D], F32, name="rso")
            nc.sync.dma_start(
                out=rso,
                in_=rs_out[2 * b + tt // 2][(tt % 2) * 128:(tt % 2 + 1) * 128, :])
            nc.vector.tensor_add(z_sb[:, zi, :], xr, rso)
            nc.vector.tensor_add(z_sb[:, zi, :], z_sb[:, zi, :], projb_bc)
            stats = p_st.tile([128, 6], F32, name="stats2")
            nc.vector.bn_stats(stats, z_sb[:, zi, :])
            mv = p_st.tile([128, 2], F32, name="mv2")
            nc.vector.bn_aggr(mv, stats)
            std = p_st.tile([128, 1], F32, name="std2")
            nc.scalar.activation(std, mv[:, 1:2], AF.Sqrt, bias=eps_sb, scale=1.0)
            rin = p_st.tile([128, 1], F32, name="rin2")
            nc.vector.reciprocal(rin, std)
            h2 = p_h.tile([128, D], BF16, name="h2")
            nc.vector.tensor_scalar(h2, z_sb[:, zi, :], mv[:, 0:1], rin,
                                    op0=ALU.subtract, op1=ALU.mult)
            for ft in range(FT):
                tp_ps = ps_s2.tile([128, 128], BF16, name="tp2_ps", tag="s2")
                nc.tensor.transpose(tp_ps, h2[:, ft * 128:(ft + 1) * 128], ident_bf)
                nc.vector.tensor_copy(h2T[:, ft, tt * 128:(tt + 1) * 128], tp_ps)
        return h2T

    def emit_mlp_ffn(b, h2T):
        hg = p_hg.tile([128, MT, CHUNK], BF16, name="hg", tag="hg")
        for mt in range(MT):
            hid_ps = ps_big.tile([128, CHUNK], F32, name="mhid_ps", tag="big")
            for ft in range(FT):
                nc.tensor.matmul(hid_ps, w1_sb[:, ft, mt * 128:(mt + 1) * 128],
                                 h2T[:, ft, :], start=(ft == 0), stop=(ft == FT - 1))
            nc.scalar.activation(hg[:, mt, :], hid_ps, AF.Gelu,
                                 bias=b1r_sb[:, mt:mt + 1], scale=1.0)
        outT = p_mlp.tile([128, FT, CHUNK], F32, name="outT")
        for ft in range(FT):
            o2_ps = ps_big.tile([128, CHUNK], F32, name="o2_ps", tag="big")
            for mt in range(MT):
                nc.tensor.matmul(o2_ps, w2_sb[:, mt, ft * 128:(ft + 1) * 128],
                                 hg[:, mt, :], start=(mt == 0), stop=(mt == MT - 1))
            nc.vector.tensor_copy(outT[:, ft, :], o2_ps)
        for tt in range(4):
            out_sb = p_out.tile([128, D], F32, name="out_sb")
            for ft in range(FT):
                tp_ps = ps_s2.tile([128, 128], F32, name="tpo_ps", tag="s2")
                nc.tensor.transpose(tp_ps, outT[:, ft, tt * 128:(tt + 1) * 128],
                                    ident_f32)
                csl = slice(ft * 128, (ft + 1) * 128)
                nc.vector.tensor_add(out_sb[:, csl], tp_ps, z_sb[:, b * 4 + tt, csl])
                nc.vector.tensor_add(out_sb[:, csl], out_sb[:, csl], b2_bc[:, csl])
            r0 = b * SHARD + tt * 128
            nc.sync.dma_start(out=out_ap[r0:r0 + 128, :], in_=out_sb)

    for ci in range(16):
        emit_pe(ci)
    for ci in range(8, 16):
        emit_ln_stats(ci)
    for ci in range(8):
        emit_ln_qkv(ci)
    emit_attention(0, unit_gen=lambda qc: emit_ln_qkv_units(8 + qc))
    state = {}
    emit_attention(1, {
        4: lambda: state.update(h2T0=emit_mlp_ln(0)),
        6: lambda: emit_mlp_ffn(0, state["h2T0"]),
    })
    h2T1 = emit_mlp_ln(1)
    emit_mlp_ffn(1, h2T1)

    # ACT table-set phase ordering: gelus (phase A) -> sqrts (LN1) -> exps
    ga, sh, e0 = (act_groups["gelu_a"], act_groups["sqrt_head"],
                  act_groups["exp0"])
    add_dep_helper(sh[0].ins, ga[-1].ins, sync=True,
                   reason="act-set: sqrt after all gelu")
    add_dep_helper(e0[0].ins, sh[-1].ins, sync=True,
                   reason="act-set: exp after all sqrt")

    for p in reversed(ctx_pools):
        p.__exit__(None, None, None)


_INPUT_SPECS = {
    "x": ([NT, D], F32),
    "xs": ([2 * SHARD, D], F32),
    "posT": ([3, NT], BF16),
    "wqk": ([128, FT, 128], BF16),
    "wv": ([128, FT, 64], BF16),
    "bq": ([64, 1], F32),
    "bk": ([64, 1], F32),
    "bv": ([64, 1], F32),
    "pe_w1": ([3, D], BF16),
    "pe_b1r": ([128, FT], F32),
    "pe_w2c": ([128, FT, 64], BF16),
    "pe_b2c": ([64, 1], F32),
    "projw": ([64, D], BF16),
    "projb": ([D], F32),
    "w1": ([128, FT, MLP_HID], BF16),
    "b1r": ([128, MT], F32),
    "w2": ([128, MT, D], BF16),
    "b2": ([D], F32),
}

_NC_CACHE = []


def build_module():
    if _NC_CACHE:
        return _NC_CACHE[0]
    nc = bacc.Bacc("TRN2", target_bir_lowering=False, debug=False,
                   num_devices=N_CORES)
    io = {}
    for name, (shape, dt) in _INPUT_SPECS.items():
        io[name] = nc.dram_tensor(name, shape, dt, kind="ExternalInput").ap()
    io["out"] = nc.dram_tensor("out", [2 * SHARD, D], F32,
                               kind="ExternalOutput").ap()
    with tile.TileContext(nc) as tc:
        _emit(nc, tc, io)
    nc.compile()
    _NC_CACHE.append(nc)
    return nc


def _feat_tiles(a, inner):
    """[D_total, inner] -> [128, D_total//128, inner] (partition-major tiles)."""
    d = a.shape[0]
    return np.ascontiguousarray(
        a.reshape(d // 128, 128, inner).transpose(1, 0, 2))


def _prep_inputs(c, x, pos, qkv_w, qkv_b, proj_w, proj_b, pe_w1, pe_b1, pe_w2,
                 pe_b2, mlp_w1, mlp_b1, mlp_w2, mlp_b2, n1_g, n1_b, n2_g, n2_b):
    bf = ml_dtypes.bfloat16
    f32 = np.float32
    x_flat = np.ascontiguousarray(x.reshape(NT, D).astype(f32))
    cs = slice(c * HD, (c + 1) * HD)
    g1 = n1_g.astype(np.float64)[:, None]
    wq = qkv_w[:, cs] * g1
    wk = qkv_w[:, D + c * HD:D + (c + 1) * HD] * g1
    wv = qkv_w[:, 2 * D + c * HD:2 * D + (c + 1) * HD] * g1
    bq_f = qkv_b[cs] + n1_b @ qkv_w[:, cs]
    bk_f = qkv_b[D + c * HD:D + (c + 1) * HD] + n1_b @ qkv_w[:, D + c * HD:D + (c + 1) * HD]
    bv_f = qkv_b[2 * D + c * HD:2 * D + (c + 1) * HD] + n1_b @ qkv_w[:, 2 * D + c * HD:2 * D + (c + 1) * HD]
    w1_f = mlp_w1 * n2_g.astype(np.float64)[:, None]
    b1_f = mlp_b1 + n2_b @ mlp_w1
    xs = np.concatenate(
        [x_flat[b * N + (tt // 2) * (N // 2) + c * 256 + (tt % 2) * 128:][:128]
         for b in range(B) for tt in range(4)], axis=0)
    per_part = lambda v: np.ascontiguousarray(
        v.reshape(-1, 128).T.astype(f32))
    return {
        "x": x_flat,
        "xs": np.ascontiguousarray(xs),
        "posT": np.ascontiguousarray(pos.reshape(NT, 3).T.astype(bf)),
        "wqk": _feat_tiles(np.concatenate([wq, wk], axis=1).astype(bf), 128),
        "wv": _feat_tiles(wv.astype(bf), HD),
        "bq": bq_f.astype(f32).reshape(HD, 1),
        "bk": bk_f.astype(f32).reshape(HD, 1),
        "bv": bv_f.astype(f32).reshape(HD, 1),
        "pe_w1": np.ascontiguousarray(pe_w1.astype(bf)),
        "pe_b1r": per_part(pe_b1),
        "pe_w2c": _feat_tiles(pe_w2[:, cs].astype(bf), HD),
        "pe_b2c": pe_b2[cs].astype(f32).reshape(HD, 1),
        "projw": np.ascontiguousarray(proj_w[cs, :].astype(bf)),
        "projb": proj_b.astype(f32),
        "w1": _feat_tiles(w1_f.astype(bf), MLP_HID),
        "b1r": per_part(b1_f),
        "w2": _feat_tiles(mlp_w2.astype(bf), D),
        "b2": mlp_b2.astype(f32),
    }


def kernel(**inputs):
    nc = build_module()
    in_maps = [_prep_inputs(c, **{k: np.asarray(v) for k, v in inputs.items()})
               for c in range(N_CORES)]
    res = run_bass_kernel_spmd(nc, in_maps, core_ids=list(range(N_CORES)),
                               trace=False)
    out = np.empty((B, N, D), np.float32)
    for c in range(N_CORES):
        o = res.results[c]["out"]
        for b in range(B):
            for tt in range(4):
                g0 = (tt // 2) * (N // 2) + c * 256 + (tt % 2) * 128
                out[b, g0:g0 + 128, :] = o[b * SHARD + tt * 128:][:128, :]
    return out


# revision 23
# speedup vs baseline: 1.1580x; 1.0025x over previous
"""PointTransformerLayer on 8 Trainium2 NeuronCores.

Sharding: head-parallel attention (H=8, core c owns head c for both batches),
megatron-style row-sharded proj with a ReduceScatter over cores, then
token-sharded MLP (each core gets 512 tokens of each batch).

Layouts: all activations feeding matmuls are kept feature-major ("transposed",
[feat, token]) so the PE contracts over features; LayerNorm stats are computed
token-major and the normalize+scale is folded into the PE-transpose epilogue.
Softmax skips max-subtraction (max logit ~38.6, exp fits fp32 easily) and gets
its denominators for free from a ones-column appended to V.
"""

import numpy as np
import ml_dtypes

import concourse.bass as bass
import concourse.tile as tile
from concourse.tile import add_dep_helper
from concourse import bacc, masks, mybir
from concourse.bass_utils import run_bass_kernel_spmd

F32 = mybir.dt.float32
BF16 = mybir.dt.bfloat16
AF = mybir.ActivationFunctionType
ALU = mybir.AluOpType

N_CORES = 8
B, N, D = 2, 4096, 512
H, HD = 8, 64
SCALE = HD ** -0.5
MLP_HID = 4 * D
EPS = 1e-5
NT = B * N              # 8192 tokens total
CHUNK = 512             # token chunk for projections / q-blocks
NCH = NT // CHUNK       # 16 chunks (8 per batch)
KT = N // 128           # 32 k-tiles per batch
FT = D // 128           # 4 feature tiles
MT = MLP_HID // 128     # 16 mlp hidden tiles
SHARD = N // N_CORES    # 512 tokens per batch per core


def _emit(nc, tc, io):
    ctx_pools = []
    act_groups = {"gelu_a": [], "sqrt_head": [], "exp0": []}

    def pool(name, bufs, space="SBUF"):
        p = tc.tile_pool(name=name, bufs=bufs, space=space)
        ctx_pools.append(p)
        return p.__enter__()

    consts = pool("consts", 1)
    persist = pool("persist", 1)
    dram = pool("dram", 1, "DRAM")
    p_x = pool("p_x", 2)
    p_st = pool("p_st", 4)
    p_h = pool("p_h", 4)
    p_hT = pool("p_hT", 2)
    p_hidT = pool("p_hidT", 2)
    p_qkb = pool("p_qkb", 2)
    p_vT = pool("p_vT", 2)
    p_pT = pool("p_pT", 3)
    p_oT = pool("p_oT", 2)
    p_pr = pool("p_pr", 2)
    p_mlp = pool("p_mlp", 2)
    p_hg = pool("p_hg", 1)
    p_out = pool("p_out", 2)
    ps_big = pool("ps_big", 2, "PSUM")
    ps_acc = pool("ps_acc", 2, "PSUM")
    ps_s2 = pool("ps_s2", 2, "PSUM")

    # ---- constants / weights into SBUF ----
    ident_bf = consts.tile([128, 128], BF16)
    masks.make_identity(nc, ident_bf)
    ident_f32 = consts.tile([128, 128], F32)
    masks.make_identity(nc, ident_f32)
    ones64 = consts.tile([1, 64], F32)
    nc.vector.memset(ones64, 1.0)
    eps_sb = consts.tile([128, 1], F32)
    nc.vector.memset(eps_sb, EPS)

    def load(name, shape, dtype):
        t = consts.tile(shape, dtype, name=f"c_{name}")
        nc.sync.dma_start(out=t, in_=io[name])
        return t

    wqk_sb = load("wqk", [128, FT, 128], BF16)
    wv_sb = load("wv", [128, FT, 64], BF16)
    bq_sb = load("bq", [64, 1], F32)
    bk_sb = load("bk", [64, 1], F32)
    bv_sb = load("bv", [64, 1], F32)
    pe_w1_sb = load("pe_w1", [3, D], BF16)
    pe_b1_sb = load("pe_b1r", [128, FT], F32)
    pe_w2_sb = load("pe_w2c", [128, FT, 64], BF16)
    pe_b2_sb = load("pe_b2c", [64, 1], F32)
    projw_sb = load("projw", [64, D], BF16)
    w1_sb = load("w1", [128, FT, MLP_HID], BF16)
    b1r_sb = load("b1r", [128, MT], F32)
    w2_sb = load("w2", [128, MT, D], BF16)
    posT_sb = p_hg.tile([3, NT], BF16, name="c_posT", tag="hg")
    nc.sync.dma_start(out=posT_sb, in_=io["posT"])
    xs_ap = io["xs"]
    x_ap = io["x"]
    out_ap = io["out"]

    def part_bcast(ap, p=128):
        return bass.AP(tensor=ap.tensor, offset=ap.offset,
                       ap=[[0, p], *[list(d) for d in ap.ap]])

    projb_bc = consts.tile([128, D], F32)
    nc.sync.dma_start(out=projb_bc, in_=part_bcast(io["projb"]))
    b2_bc = consts.tile([128, D], F32)
    nc.sync.dma_start(out=b2_bc, in_=part_bcast(io["b2"]))

    # ---- persistent activations ----
    qT_sb = persist.tile([64, NT], BF16)
    kT_sb = persist.tile([64, NT], BF16)
    V_sb = persist.tile([128, NT // 128, 65], BF16)  # token-major V + ones col
    z_sb = persist.tile([128, 8, D], F32)            # post-attn residual (shard)
    mv_b1 = persist.tile([128, 32, 2], F32)          # hoisted LN1 stats, batch 1
    rin_b1 = persist.tile([128, 32], F32)
    nc.vector.memset(V_sb[:, :, 64:65], 1.0)

    rs_in = [dram.tile([N // 2, D], F32, name=f"rs_in{i}") for i in range(2 * B)]
    rs_out = [dram.tile([N // 2 // N_CORES, D], F32, name=f"rs_out{i}")
              for i in range(2 * B)]

    # ============ stage 1a: positional embedding (all Gelu up front) =========
    def emit_pe(ci):
        sl = slice(ci * CHUNK, (ci + 1) * CHUNK)
        hidT = p_hidT.tile([128, FT, CHUNK], BF16, name="hidT")
        for ft in range(FT):
            hid_ps = ps_big.tile([128, CHUNK], F32, name="hid_ps", tag="big")
            nc.tensor.matmul(hid_ps, pe_w1_sb[:, ft * 128:(ft + 1) * 128],
                             posT_sb[:, sl], start=True, stop=True)
            act_groups["gelu_a"].append(
                nc.scalar.activation(hidT[:, ft, :], hid_ps, AF.Gelu,
                                     bias=pe_b1_sb[:, ft:ft + 1], scale=1.0))
        peT_ps = ps_acc.tile([64, CHUNK], F32, name="peT_ps", tag="acc")
        for ft in range(FT):
            nc.tensor.matmul(peT_ps, pe_w2_sb[:, ft, :], hidT[:, ft, :],
                             start=(ft == 0), stop=(ft == FT - 1))
        nc.vector.tensor_scalar_add(qT_sb[:, sl], peT_ps, pe_b2_sb)
        nc.vector.tensor_scalar_add(kT_sb[:, sl], peT_ps, pe_b2_sb)

    # ============ stage 1b-pre: LN1 stats only (hoists ACT Sqrt) =============
    def emit_ln_stats(ci):
        for tt in range(4):
            g = ci * 4 + tt
            gi = g - 32
            x_sb = p_x.tile([128, D], F32, name="x_sb")
            nc.sync.dma_start(out=x_sb, in_=x_ap[g * 128:(g + 1) * 128, :])
            stats = p_st.tile([128, 6], F32, name="stats")
            nc.vector.bn_stats(stats, x_sb)
            nc.vector.bn_aggr(mv_b1[:, gi, :], stats)
            std = p_st.tile([128, 1], F32, name="std")
            act_groups["sqrt_head"].append(
                nc.scalar.activation(std, mv_b1[:, gi, 1:2], AF.Sqrt,
                                     bias=eps_sb, scale=1.0))
            nc.vector.reciprocal(rin_b1[:, gi:gi + 1], std)

    # ============ stage 1b: LN1 + transpose + qkvT ===========================
    # Emits DVE/DMA prep now; returns PE-work thunks for fine-grained
    # interleaving into the attention k-loop (keeps PE duty high for HAM).
    def emit_ln_qkv_units(ci):
        sl = slice(ci * CHUNK, (ci + 1) * CHUNK)
        hT = p_hT.tile([128, FT, CHUNK], BF16, name="hT")
        h_tiles = []
        for tt in range(4):
            g = ci * 4 + tt
            x_sb = p_x.tile([128, D], F32, name="x_sb")
            nc.sync.dma_start(out=x_sb, in_=x_ap[g * 128:(g + 1) * 128, :])
            if ci < 8:
                stats = p_st.tile([128, 6], F32, name="stats")
                nc.vector.bn_stats(stats, x_sb)
                mv = p_st.tile([128, 2], F32, name="mv")
                nc.vector.bn_aggr(mv, stats)
                std = p_st.tile([128, 1], F32, name="std")
                act_groups["sqrt_head"].append(
                    nc.scalar.activation(std, mv[:, 1:2], AF.Sqrt, bias=eps_sb,
                                         scale=1.0))
                rin = p_st.tile([128, 1], F32, name="rin")
                nc.vector.reciprocal(rin, std)
                mu, ri = mv[:, 0:1], rin
            else:
                gi = g - 32
                mu, ri = mv_b1[:, gi, 0:1], rin_b1[:, gi:gi + 1]
            h_sb = p_h.tile([128, D], BF16, name="h_sb")
            nc.vector.tensor_scalar(h_sb, x_sb, mu, ri,
                                    op0=ALU.subtract, op1=ALU.mult)
            h_tiles.append(h_sb)

        def tp_unit(tt, h_sb):
            tp_ps = ps_s2.tile([128, FT, 128], BF16, name="tp_ps", tag="s2")
            for ft in range(FT):
                nc.tensor.transpose(tp_ps[:, ft, :],
                                    h_sb[:, ft * 128:(ft + 1) * 128], ident_bf)
            nc.vector.tensor_copy(hT[:, :, tt * 128:(tt + 1) * 128], tp_ps)

        def qk_unit():
            qk_ps = ps_big.tile([128, CHUNK], F32, name="qk_ps", tag="big")
            for ft in range(FT):
                nc.tensor.matmul(qk_ps, wqk_sb[:, ft, :], hT[:, ft, :],
                                 start=(ft == 0), stop=(ft == FT - 1))
            bq_tmp = p_qkb.tile([64, CHUNK], BF16, name="bq_tmp", tag="qkb")
            nc.vector.tensor_scalar_add(bq_tmp, qk_ps[0:64, :], bq_sb)
            nc.vector.tensor_add(qT_sb[:, sl], qT_sb[:, sl], bq_tmp)
            bk_tmp = p_qkb.tile([64, CHUNK], BF16, name="bk_tmp", tag="qkb")
            nc.vector.tensor_scalar_add(bk_tmp, qk_ps[64:128, :], bk_sb)
            nc.vector.tensor_add(kT_sb[:, sl], kT_sb[:, sl], bk_tmp)

        def v_unit():
            v_ps = ps_acc.tile([64, CHUNK], F32, name="v_ps", tag="acc")
            for ft in range(FT):
                nc.tensor.matmul(v_ps, wv_sb[:, ft, :], hT[:, ft, :],
                                 start=(ft == 0), stop=(ft == FT - 1))
            vT = p_vT.tile([64, CHUNK], BF16, name="vT")
            nc.vector.tensor_scalar_add(vT, v_ps, bv_sb)
            for ss in range(4):
                vtp_ps = ps_s2.tile([128, 64], BF16, name="vtp_ps", tag="s2")
                nc.tensor.transpose(vtp_ps, vT[:, ss * 128:(ss + 1) * 128],
                                    ident_bf[0:64, 0:64])
                nc.vector.tensor_copy(V_sb[:, ci * 4 + ss, 0:64], vtp_ps)

        units = [lambda tt=tt, h=h_tiles[tt]: tp_unit(tt, h) for tt in range(4)]
        units += [qk_unit, v_unit]
        return units

    def emit_ln_qkv(ci):
        for u in emit_ln_qkv_units(ci):
            u()

    # ============ stage 2: attention + proj partial ==========================
    def emit_attention(b, fillers=None, unit_gen=None):
        base = b * N
        for qc in range(N // CHUNK):
            units = list(unit_gen(qc)) if unit_gen else []
            qsl = slice(base + qc * CHUNK, base + (qc + 1) * CHUNK)
            oT_ps = ps_acc.tile([65, CHUNK], F32, name="oT_ps", tag="acc")
            for kp in range(KT // 2):
                st2_ps = ps_s2.tile([128, 2 * CHUNK], F32, name="st2_ps", tag="s2")
                for j in range(2):
                    kt = kp * 2 + j
                    ksl = slice(base + kt * 128, base + (kt + 1) * 128)
                    nc.tensor.matmul(st2_ps[:, j * CHUNK:(j + 1) * CHUNK],
                                     kT_sb[:, ksl], qT_sb[:, qsl],
                                     start=True, stop=True)
                pT = p_pT.tile([128, 2 * CHUNK], BF16, name="pT")
                e = nc.scalar.activation(pT, st2_ps, AF.Exp, scale=SCALE)
                if b == 0:
                    act_groups["exp0"].append(e)
                for j in range(2):
                    kt = kp * 2 + j
                    nc.tensor.matmul(oT_ps, V_sb[:, b * KT + kt, :],
                                     pT[:, j * CHUNK:(j + 1) * CHUNK],
                                     start=(kt == 0), stop=(kt == KT - 1))
                if kp % 2 == 1 and units:
                    units.pop(0)()
            o65_sb = p_oT.tile([65, CHUNK], F32, name="o65_sb", tag="o65")
            nc.vector.tensor_copy(o65_sb, oT_ps)
            dcol_ps = ps_big.tile([128, 4], F32, name="dcol_ps", tag="big")
            for ss in range(4):
                nc.tensor.transpose(dcol_ps[:, ss:ss + 1],
                                    o65_sb[64:65, ss * 128:(ss + 1) * 128],
                                    ident_f32[64:65, 64:65])
            rcol_sb = p_st.tile([128, 4], F32, name="rcol_sb")
            nc.vector.reciprocal(rcol_sb, dcol_ps)
            rrow_ps = ps_big.tile([1, CHUNK], F32, name="rrow_ps", tag="big")
            for ss in range(4):
                nc.tensor.transpose(rrow_ps[:, ss * 128:(ss + 1) * 128],
                                    rcol_sb[:, ss:ss + 1], ident_f32)
            recip = p_st.tile([1, CHUNK], F32, name="recip")
            nc.vector.tensor_copy(recip, rrow_ps)
            rb_ps = ps_big.tile([64, CHUNK], F32, name="rb_ps", tag="big")
            nc.tensor.matmul(rb_ps, ones64, recip, start=True, stop=True)
            oT_sb = p_oT.tile([64, CHUNK], BF16, name="oT_sb")
            nc.vector.tensor_mul(oT_sb, o65_sb[0:64, :], rb_ps)
            for ss in range(4):
                pr_ps = ps_big.tile([128, D], F32, name="pr_ps", tag="big")
                nc.tensor.matmul(pr_ps, oT_sb[:, ss * 128:(ss + 1) * 128], projw_sb,
                                 start=True, stop=True)
                pr_sb = p_pr.tile([128, D], F32, name="pr_sb")
                nc.vector.tensor_copy(pr_sb, pr_ps)
                r0 = (qc % 4) * CHUNK + ss * 128
                nc.sync.dma_start(out=rs_in[2 * b + qc // 4][r0:r0 + 128, :],
                                  in_=pr_sb)
            if qc == 3 or qc == 7:
                half = qc // 4
                nc.gpsimd.collective_compute(
                    "ReduceScatter", ALU.add,
                    replica_groups=[list(range(N_CORES))],
                    ins=[rs_in[2 * b + half].opt()],
                    outs=[rs_out[2 * b + half].opt()],
                )
            if fillers and qc in fillers:
                fillers[qc]()

    # =================== stage 3: residual + LN2 + MLP (token shard) =========
    def emit_mlp_ln(b):
        h2T = p_mlp.tile([128, FT, CHUNK], BF16, name="h2T")
        for tt in range(4):
            zi = b * 4 + tt
            xr = p_x.tile([128, D], F32, name="xr")
            nc.sync.dma_start(out=xr, in_=xs_ap[zi * 128:(zi + 1) * 128, :])
            rso = p_pr.tile([128, D], F32, name="rso")
            nc.sync.dma_start(
                out=rso,
                in_=rs_out[2 * b + tt // 2][(tt % 2) * 128:(tt % 2 + 1) * 128, :])
            nc.vector.tensor_add(z_sb[:, zi, :], xr, rso)
            nc.vector.tensor_add(z_sb[:, zi, :], z_sb[:, zi, :], projb_bc)
            stats = p_st.tile([128, 6], F32, name="stats2")
            nc.vector.bn_stats(stats, z_sb[:, zi, :])
            mv = p_st.tile([128, 2], F32, name="mv2")
            nc.vector.bn_aggr(mv, stats)
            std = p_st.tile([128, 1], F32, name="std2")
            nc.scalar.activation(std, mv[:, 1:2], AF.Sqrt, bias=eps_sb, scale=1.0)
            rin = p_st.tile([128, 1], F32, name="rin2")
            nc.vector.reciprocal(rin, std)
            h2 = p_h.tile([128, D], BF16, name="h2")
            nc.vector.tensor_scalar(h2, z_sb[:, zi, :], mv[:, 0:1], rin,
                                    op0=ALU.subtract, op1=ALU.mult)
            tp_ps = ps_s2.tile([128, FT, 128], BF16, name="tp2_ps", tag="s2")
            for ft in range(FT):
                nc.tensor.transpose(tp_ps[:, ft, :],
                                    h2[:, ft * 128:(ft + 1) * 128], ident_bf)
            nc.vector.tensor_copy(h2T[:, :, tt * 128:(tt + 1) * 128], tp_ps)
        return h2T

    def emit_mlp_ffn(b, h2T):
        hg = p_hg.tile([128, MT, CHUNK], BF16, name="hg", tag="hg")
        for mt in range(MT):
            hid_ps = ps_big.tile([128, CHUNK], F32, name="mhid_ps", tag="big")
            for ft in range(FT):
                nc.tensor.matmul(hid_ps, w1_sb[:, ft, mt * 128:(mt + 1) * 128],
                                 h2T[:, ft, :], start=(ft == 0), stop=(ft == FT - 1))
            nc.scalar.activation(hg[:, mt, :], hid_ps, AF.Gelu,
                                 bias=b1r_sb[:, mt:mt + 1], scale=1.0)
        outT = p_mlp.tile([128, FT, CHUNK], F32, name="outT")
        for ft in range(FT):
            o2_ps = ps_big.tile([128, CHUNK], F32, name="o2_ps", tag="big")
            for mt in range(MT):
                nc.tensor.matmul(o2_ps, w2_sb[:, mt, ft * 128:(ft + 1) * 128],
                                 hg[:, mt, :], start=(mt == 0), stop=(mt == MT - 1))
            nc.vector.tensor_copy(outT[:, ft, :], o2_ps)
        for tt in range(4):
            out_sb = p_out.tile([128, D], F32, name="out_sb")
            tp_ps = ps_s2.tile([128, FT, 128], F32, name="tpo_ps", tag="s2")
            for ft in range(FT):
                nc.tensor.transpose(tp_ps[:, ft, :],
                                    outT[:, ft, tt * 128:(tt + 1) * 128], ident_f32)
            out_v = out_sb.rearrange("p (f t) -> p f t", f=FT)
            nc.vector.tensor_add(out_v, tp_ps,
                                 z_sb[:, b * 4 + tt, :].rearrange(
                                     "p (f t) -> p f t", f=FT))
            nc.vector.tensor_add(out_sb, out_sb, b2_bc)
            r0 = b * SHARD + tt * 128
            nc.sync.dma_start(out=out_ap[r0:r0 + 128, :], in_=out_sb)

    for ci in range(16):
        emit_pe(ci)
    for ci in range(8, 16):
        emit_ln_stats(ci)
    for ci in range(8):
        emit_ln_qkv(ci)
    emit_attention(0, unit_gen=lambda qc: emit_ln_qkv_units(8 + qc))
    state = {}
    emit_attention(1, {
        4: lambda: state.update(h2T0=emit_mlp_ln(0)),
        6: lambda: emit_mlp_ffn(0, state["h2T0"]),
    })
    h2T1 = emit_mlp_ln(1)
    emit_mlp_ffn(1, h2T1)

    # ACT table-set phase ordering: gelus (phase A) -> sqrts (LN1) -> exps
    ga, sh, e0 = (act_groups["gelu_a"], act_groups["sqrt_head"],
                  act_groups["exp0"])
    add_dep_helper(sh[0].ins, ga[-1].ins, sync=True,
                   reason="act-set: sqrt after all gelu")
    add_dep_helper(e0[0].ins, sh[-1].ins, sync=True,
                   reason="act-set: exp after all sqrt")

    for p in reversed(ctx_pools):
        p.__exit__(None, None, None)


_INPUT_SPECS = {
    "x": ([NT, D], F32),
    "xs": ([2 * SHARD, D], F32),
    "posT": ([3, NT], BF16),
    "wqk": ([128, FT, 128], BF16),
    "wv": ([128, FT, 64], BF16),
    "bq": ([64, 1], F32),
    "bk": ([64, 1], F32),
    "bv": ([64, 1], F32),
    "pe_w1": ([3, D], BF16),
    "pe_b1r": ([128, FT], F32),
    "pe_w2c": ([128, FT, 64], BF16),
    "pe_b2c": ([64, 1], F32),
    "projw": ([64, D], BF16),
    "projb": ([D], F32),
    "w1": ([128, FT, MLP_HID], BF16),
    "b1r": ([128, MT], F32),
    "w2": ([128, MT, D], BF16),
    "b2": ([D], F32),
}

_NC_CACHE = []


def build_module():
    if _NC_CACHE:
        return _NC_CACHE[0]
    nc = bacc.Bacc("TRN2", target_bir_lowering=False, debug=False,
                   num_devices=N_CORES)
    io = {}
    for name, (shape, dt) in _INPUT_SPECS.items():
        io[name] = nc.dram_tensor(name, shape, dt, kind="ExternalInput").ap()
    io["out"] = nc.dram_tensor("out", [2 * SHARD, D], F32,
                               kind="ExternalOutput").ap()
    with tile.TileContext(nc) as tc:
        _emit(nc, tc, io)
    nc.compile()
    _NC_CACHE.append(nc)
    return nc


def _feat_tiles(a, inner):
    """[D_total, inner] -> [128, D_total//128, inner] (partition-major tiles)."""
    d = a.shape[0]
    return np.ascontiguousarray(
        a.reshape(d // 128, 128, inner).transpose(1, 0, 2))


def _prep_inputs(c, x, pos, qkv_w, qkv_b, proj_w, proj_b, pe_w1, pe_b1, pe_w2,
                 pe_b2, mlp_w1, mlp_b1, mlp_w2, mlp_b2, n1_g, n1_b, n2_g, n2_b):
    bf = ml_dtypes.bfloat16
    f32 = np.float32
    x_flat = np.ascontiguousarray(x.reshape(NT, D).astype(f32))
    cs = slice(c * HD, (c + 1) * HD)
    g1 = n1_g.astype(np.float64)[:, None]
    wq = qkv_w[:, cs] * g1
    wk = qkv_w[:, D + c * HD:D + (c + 1) * HD] * g1
    wv = qkv_w[:, 2 * D + c * HD:2 * D + (c + 1) * HD] * g1
    bq_f = qkv_b[cs] + n1_b @ qkv_w[:, cs]
    bk_f = qkv_b[D + c * HD:D + (c + 1) * HD] + n1_b @ qkv_w[:, D + c * HD:D + (c + 1) * HD]
    bv_f = qkv_b[2 * D + c * HD:2 * D + (c + 1) * HD] + n1_b @ qkv_w[:, 2 * D + c * HD:2 * D + (c + 1) * HD]
    w1_f = mlp_w1 * n2_g.astype(np.float64)[:, None]
    b1_f = mlp_b1 + n2_b @ mlp_w1
    xs = np.concatenate(
        [x_flat[b * N + (tt // 2) * (N // 2) + c * 256 + (tt % 2) * 128:][:128]
         for b in range(B) for tt in range(4)], axis=0)
    per_part = lambda v: np.ascontiguousarray(
        v.reshape(-1, 128).T.astype(f32))
    return {
        "x": x_flat,
        "xs": np.ascontiguousarray(xs),
        "posT": np.ascontiguousarray(pos.reshape(NT, 3).T.astype(bf)),
        "wqk": _feat_tiles(np.concatenate([wq, wk], axis=1).astype(bf), 128),
        "wv": _feat_tiles(wv.astype(bf), HD),
        "bq": bq_f.astype(f32).reshape(HD, 1),
        "bk": bk_f.astype(f32).reshape(HD, 1),
        "bv": bv_f.astype(f32).reshape(HD, 1),
        "pe_w1": np.ascontiguousarray(pe_w1.astype(bf)),
        "pe_b1r": per_part(pe_b1),
        "pe_w2c": _feat_tiles(pe_w2[:, cs].astype(bf), HD),
        "pe_b2c": pe_b2[cs].astype(f32).reshape(HD, 1),
        "projw": np.ascontiguousarray(proj_w[cs, :].astype(bf)),
        "projb": proj_b.astype(f32),
        "w1": _feat_tiles(w1_f.astype(bf), MLP_HID),
        "b1r": per_part(b1_f),
        "w2": _feat_tiles(mlp_w2.astype(bf), D),
        "b2": mlp_b2.astype(f32),
    }


def kernel(**inputs):
    nc = build_module()
    in_maps = [_prep_inputs(c, **{k: np.asarray(v) for k, v in inputs.items()})
               for c in range(N_CORES)]
    res = run_bass_kernel_spmd(nc, in_maps, core_ids=list(range(N_CORES)),
                               trace=False)
    out = np.empty((B, N, D), np.float32)
    for c in range(N_CORES):
        o = res.results[c]["out"]
        for b in range(B):
            for tt in range(4):
                g0 = (tt // 2) * (N // 2) + c * 256 + (tt % 2) * 128
                out[b, g0:g0 + 128, :] = o[b * SHARD + tt * 128:][:128, :]
    return out
